# revision 46
# baseline (speedup 1.0000x reference)
"""nn_HashMapper Trainium2 kernel (8 NeuronCores, Bass/Tile) — v8.

Contract: kernel(**inputs) takes the FULL unsharded inputs
(bits [32768,1024] i32, tables [3,1024,16384] f32, positions [3,14] i32)
and returns the FULL output [32768,1024] u8.

Sharding (hardcoded): neurons j (1024) split across 8 cores (128 each) so
tables are read exactly once system-wide; every core computes the full
batch's hash addresses locally from a replicated 42-column slice of bits
(no cross-device communication at all).

Architecture — 2-bit packed tables, bf16 byte-view inputs:
  - tslice ships as the high 2 bytes of each f32 (exact for 0/1 values).
  - Each core packs its 128 neurons' table values into 2-bit fields of u16
    words via PE matmuls (8 neurons/word, 16 words = 32B per address);
    packed rows are padded to 256B in DRAM and gathered bitcast to int64
    (elem 32 x i64, the cheapest legal gather shape).
  - Addresses: bitsel packs the 42 selected bit-columns for each half of
    the batch into partitions [0:42]/[64:106]; PE matmuls produce all
    32768 addresses; a wrapped DRAM round-trip (split per batch half so
    the first half unlocks early) builds the replicated index tiles.
  - Votes: gathered words of the 3 hash tables are summed as u16 (2-bit
    fields hold 0..3, no carries); majority = bit1 of each field,
    extracted with 4 tensor_scalar (shift, and-0x0101) passes emitting two
    output bytes per u16; the host inverse-permutes output columns.
  - Engine layout: SP: consts | interleaved tslice tiles | wrapped addr
    writes | PT1 | half the out writes.  ACT: interleaved tslice tiles |
    PT0/PT2 | idx reads | other out writes.  Pool: bitsel cast-loads |
    gathers.  DVE: addr/psum copies | votes + unpack.  PE: all matmuls.
"""

from contextlib import ExitStack

import numpy as np

import concourse.bass as bass
import concourse.bacc as bacc
import concourse.tile as tile
import concourse.mybir as mybir
from concourse.bass_utils import run_bass_kernel_spmd

F32 = mybir.dt.float32
BF16 = mybir.dt.bfloat16
I32 = mybir.dt.int32
I16 = mybir.dt.int16
U16 = mybir.dt.uint16
I64 = mybir.dt.int64
U8 = mybir.dt.uint8
A = mybir.AluOpType

N_BITS = 1024
NE = 16384
H = 3
JS = 128
B_TOTAL = 32768
N_CORES = 8
NSEL = H * 14  # 42 selected bit columns

NW = 16         # u16 words per packed row (8 neurons each, 2-bit fields)
ROW_U16 = 128   # padded row size in u16 (256B, gather elem floor)
CHUNK = 4096    # gather chunk (batch rows per dma_gather)
CC = CHUNK // 128
NCK = B_TOTAL // CHUNK  # 8
NBT = B_TOTAL // 128    # 256 address blocks
GC = B_TOTAL // 16      # 2048 wrapped idx columns


def _build(n_cores=N_CORES, nq=4):
    nc = bacc.Bacc(
        "TRN2", target_bir_lowering=False, num_devices=n_cores, num_swdge_queues=nq
    )
    tslice = nc.dram_tensor("tslice", [H, JS, NE], BF16, kind="ExternalInput")
    bitsel = nc.dram_tensor("bitsel", [128, B_TOTAL // 2], I32, kind="ExternalInput")
    waddr = nc.dram_tensor("waddr", [128, H], BF16, kind="ExternalInput")
    wpack = nc.dram_tensor("wpack", [JS, NW], BF16, kind="ExternalInput")
    out = nc.dram_tensor("out", [B_TOTAL, JS], U8, kind="ExternalOutput")

    # wrapped addresses, split per bitsel half: [h, half, 16, 1024] i16
    addrw = nc.dram_tensor("addrw", [H, 2, 16, GC // 2], I16)
    # packed table, padded rows: row (h, a) = PT[h, a, 0:NW] words + pad
    PT = nc.dram_tensor("PT", [H, NE, ROW_U16], U16)

    with tile.TileContext(nc) as tc, ExitStack() as ctx:
        const = ctx.enter_context(tc.tile_pool(name="const", bufs=1))
        psT = ctx.enter_context(tc.tile_pool(name="psT", bufs=4, space="PSUM"))
        psA = ctx.enter_context(tc.tile_pool(name="psA", bufs=2, space="PSUM"))
        sbT = ctx.enter_context(tc.tile_pool(name="sbT", bufs=2))
        sbG = ctx.enter_context(tc.tile_pool(name="sbG", bufs=1))

        wpk = const.tile([JS, NW], BF16)
        nc.sync.dma_start(wpk[:, :], wpack[:, :])
        wad = const.tile([128, H], BF16)
        nc.sync.dma_start(wad[:, :], waddr[:, :])

        # ---- bitsel cast-load (Pool, i32 -> bf16) + address matmuls (PE),
        #      pipelined per column half (m = 2*blk + g, so column half ch
        #      yields addr16 columns [ch*128, (ch+1)*128)); copies on Pool ----
        bsl = const.tile([128, B_TOTAL // 2], BF16)
        addr16 = const.tile([128, NBT, H], I16)
        for ch in range(2):
            if ch == 0:
                nc.gpsimd.dma_start(
                    bsl[:, 0 : B_TOTAL // 4],
                    bitsel[:, 0 : B_TOTAL // 4],
                )
            else:
                # two pieces so Pool has a scheduling point to slot cp-a in
                for pc in range(2):
                    o = B_TOTAL // 4 + pc * (B_TOTAL // 8)
                    nc.gpsimd.dma_start(
                        bsl[:, o : o + B_TOTAL // 8],
                        bitsel[:, o : o + B_TOTAL // 8],
                    )
            pa = psA.tile([128, NBT // 2, H], F32, tag="addr")
            for i in range(NBT // 2):
                blk, g = ch * 64 + i // 2, i % 2
                nc.tensor.matmul(
                    pa[:, i, :],
                    bsl[64 * g : 64 * g + NSEL, blk * 128 : (blk + 1) * 128],
                    wad[64 * g : 64 * g + NSEL, :],
                    start=True,
                    stop=True,
                )
            nc.gpsimd.tensor_copy(
                addr16[:, ch * (NBT // 2) : (ch + 1) * (NBT // 2), :],
                pa[:, :, :],
            )

        # wrapped-layout writeback per (h, bitsel-half).  The host orders
        # bitsel columns so matmul m's output partition p computes batch row
        # b = (p%8)*4096 + m*16 + p//8; region (h, half) holds slots
        # [r=p//8][(p%8)*128 + m%128] so each half lands as soon as its
        # bitsel half is processed.
        def wrap_write(eng, h, half):
            eng.dma_start(
                bass.AP(
                    addrw,
                    (h * 2 + half) * 16 * (GC // 2),
                    [[GC // 2, 16], [128, 8], [1, 128]],
                ),
                addr16[:, half * 128 : (half + 1) * 128, h],
            )

        idxs = [const.tile([128, GC], I16, tag=f"idx{h}", name=f"idx{h}")
                for h in range(H)]

        def idx_read(eng, h, half):
            eng.dma_start(
                idxs[h][:, half * (GC // 2) : (half + 1) * (GC // 2)],
                bass.AP(
                    addrw,
                    (h * 2 + half) * 16 * (GC // 2),
                    [[0, 8], [GC // 2, 16], [1, GC // 2]],
                ),
            )

        # ---- tslice load + pack (PE matmuls) + PT writes ----
        # Emission order is engine-queue order; interleave the addr-path
        # DMAs at queue positions where their dependencies are ready.
        TL = 2048  # tslice tile columns (addr)
        NT = NE // TL  # 8 tiles per h
        pts = [const.tile([128, NE // 128, NW], U16, name=f"pts{h}") for h in range(H)]

        def load_tile(h, t, eng):
            tsl = sbT.tile([128, TL], BF16, tag=f"tsl{t % 2}", bufs=3,
                           name=f"tsl_{h}_{t}")
            eng.dma_start(tsl[:, :], tslice[h, :, t * TL : (t + 1) * TL])
            ps = psT.tile([128, 16, NW], F32, tag="pack")
            for b16 in range(16):
                nc.tensor.matmul(
                    ps[:, b16, :],
                    tsl[:, b16 * 128 : (b16 + 1) * 128],
                    wpk[:, :],
                    start=True,
                    stop=True,
                )
            nc.vector.tensor_copy(pts[h][:, t * 16 : t * 16 + 16, :], ps[:, :, :])

        def pt_write(h, eng, q=None):
            # quarter-granular PT writes so each lands as soon as its pack
            # copies finish (rank range q*32:(q+1)*32)
            o, n = (0, NE // 128) if q is None else (q * 32, 32)
            eng.dma_start(
                bass.AP(
                    PT,
                    h * NE * ROW_U16 + o * 128 * ROW_U16,
                    [[ROW_U16, 128], [128 * ROW_U16, n], [1, NW]],
                ),
                pts[h][:, o : o + n, :],
            )

        # SP:  c | h0e×4 | h1e×2 | wrap-h0 | wrap-h1 | h1e×2 | h2e×4 |
        #      wrap-h2 | PT2
        # ACT: h0o×4 | h1o×2 | idx-h0 | h1o×2 | PT1 | idx-h1 | h2o×4 | idx-h2
        # Pool: bitsel | PT0 | gathers
        def load_pair(h, q):
            load_tile(h, 2 * q, nc.sync)
            load_tile(h, 2 * q + 1, nc.scalar)
            pt_write(h, nc.scalar if q % 2 == 0 else nc.sync, q)

        def load_h(h):
            for q in range(NT // 2):
                load_pair(h, q)

        load_h(0)
        wrap_write(nc.sync, 0, 0)
        idx_read(nc.sync, 0, 0)
        wrap_write(nc.scalar, 1, 0)
        idx_read(nc.scalar, 1, 0)
        load_pair(1, 0)
        load_pair(1, 1)
        wrap_write(nc.scalar, 2, 0)
        idx_read(nc.scalar, 2, 0)
        wrap_write(nc.sync, 0, 1)
        idx_read(nc.sync, 0, 1)
        load_pair(1, 2)
        load_pair(1, 3)
        wrap_write(nc.sync, 1, 1)
        idx_read(nc.sync, 1, 1)
        load_h(2)
        wrap_write(nc.scalar, 2, 1)
        idx_read(nc.scalar, 2, 1)

        # ---- gather + votes + out (hand-synced critical section) ----
        SH = [5, 4, 3]  # per-h gather slot counts
        gts = [
            [sbG.tile([128, CC, ROW_U16 // 4], I64, tag=f"g{h}s{s}", bufs=1,
                      name=f"g{h}s{s}") for s in range(SH[h])]
            for h in range(H)
        ]
        NVB = 3
        vts = [sbG.tile([128, CC, NW], U16, tag=f"vt{s}", bufs=1, name=f"vt{s}")
               for s in range(NVB)]
        ots = [sbG.tile([128, CC, 4, NW], U16, tag=f"os{s}", bufs=1, name=f"os{s}")
               for s in range(NVB)]
        def gather(h, k):
            nc.gpsimd.dma_gather(
                gts[h][k % SH[h]][:, :, :],
                bass.AP(PT, h * NE * ROW_U16, [[ROW_U16, NE], [1, ROW_U16]]).bitcast(
                    I64
                ),
                idxs[h][:, k * (CHUNK // 16) : (k + 1) * (CHUNK // 16)],
                num_idxs=CHUNK,
                num_idxs_reg=CHUNK,
                elem_size=ROW_U16 // 4,
                single_packet=False,
                queue_num=0,
            )

        # staged issue order matched to feed readiness: h0 leads, then h1,
        # then h2 with the rest interleaved (tile auto-sync handles deps and
        # slot reuse; no barrier, so gathers overlap the prep phase)
        sched = [(0, k) for k in range(SH[0])]
        sched += [(1, k) for k in range(SH[1])]
        for k in range(NCK):
            sched.append((2, k))
            if k + SH[0] < NCK:
                sched.append((0, k + SH[0]))
            if k + SH[1] < NCK:
                sched.append((1, k + SH[1]))

        gath_emitted = [[False] * NCK for _ in range(H)]
        votes_done = [0]

        def emit_votes_up_to(kmax):
            # votes + unpack + out write for chunks [votes_done, kmax)
            for k in range(votes_done[0], kmax):
                g0 = gts[0][k % SH[0]][:, :, :].bitcast(U16)[:, :, 0:NW]
                g1 = gts[1][k % SH[1]][:, :, :].bitcast(U16)[:, :, 0:NW]
                g2 = gts[2][k % SH[2]][:, :, :].bitcast(U16)[:, :, 0:NW]
                vt = vts[k % NVB]
                nc.vector.tensor_tensor(vt[:, :, :], g0, g1, op=A.add)
                nc.vector.tensor_tensor(vt[:, :, :], vt[:, :, :], g2, op=A.add)
                ot = ots[k % NVB]
                for kk in range(4):
                    nc.vector.tensor_scalar(
                        ot[:, :, kk, :], vt[:, :, :], 2 * kk + 1, 0x0101,
                        op0=A.logical_shift_right, op1=A.bitwise_and,
                    )
                eng = nc.sync if k % 2 == 0 else nc.scalar
                eng.dma_start(
                    bass.AP(out, k * CHUNK * JS, [[CC * JS, 128], [1, CC * JS]]),
                    ots[k % NVB][:, :, :, :].bitcast(U8).rearrange(
                        "p cc x w -> p (cc x w)"
                    ),
                )
            votes_done[0] = max(votes_done[0], kmax)

        for h, k in sched:
            gather(h, k)
            gath_emitted[h][k] = True
            # emit vote chains as soon as all three gathers of a chunk exist
            kk = votes_done[0]
            while kk < NCK and all(gath_emitted[h2][kk] for h2 in range(H)):
                emit_votes_up_to(kk + 1)
                kk += 1
        emit_votes_up_to(NCK)

    nc.compile()
    return nc


def _make_weights(positions):
    """Host-side tiny weight tensors derived from positions."""
    import ml_dtypes

    waddr = np.zeros((128, H), dtype=np.float32)
    for h in range(H):
        for k in range(14):
            waddr[h * 14 + k, h] = float(1 << (13 - k))
            waddr[64 + h * 14 + k, h] = float(1 << (13 - k))
    wpack = np.zeros((JS, NW), dtype=np.float32)
    for jl in range(JS):
        wpack[jl, jl // 8] = float(4 ** (jl % 8))
    return (
        waddr.astype(ml_dtypes.bfloat16),
        wpack.astype(ml_dtypes.bfloat16),
    )


_NC_CACHE = {}


def _get_nc():
    if "nc" not in _NC_CACHE:
        _NC_CACHE["nc"] = _build()
    return _NC_CACHE["nc"]


OUT_NAMES = ["out"]


def _col_unperm():
    """Map output column j_local -> device column (k*32 + w*2 + byte)."""
    dmap = np.empty(JS, dtype=np.int64)
    for jl in range(JS):
        w, r = jl // 8, jl % 8
        k, b = r % 4, r // 4
        dmap[jl] = k * 32 + w * 2 + b
    return dmap


def _make_in_maps(inputs):
    import ml_dtypes

    bits = np.asarray(inputs["bits"], dtype=np.int32)
    tables = np.ascontiguousarray(np.asarray(inputs["tables"], dtype=np.float32))
    positions = np.asarray(inputs["positions"], dtype=np.int32)
    wa, wp = _make_weights(positions)
    cols = np.array(
        [N_BITS - 1 - positions[h, k] for h in range(H) for k in range(14)],
        dtype=np.int64,
    )
    # bf16 byte-view of the f32 tables (exact: values are 0.0/1.0)
    tb16 = tables.view(np.uint16)[:, :, 1::2]
    # bitsel: 42 selected columns; device column q = blk*128 + p carries
    # rows [0:42] for m = 2*blk (partition group 0) and rows [64:106] for
    # m = 2*blk + 1 (group 1); matmul m partition p computes batch row
    # b = (p%8)*4096 + m*16 + p//8 (see wrap_write)
    sel = bits[:, cols].T  # [42, B_TOTAL]
    blk_ = np.arange(128)[:, None]
    p_ = np.arange(128)[None, :]
    b0 = (p_ % 8) * 4096 + (2 * blk_) * 16 + p_ // 8  # [128, 128]
    b1 = (p_ % 8) * 4096 + (2 * blk_ + 1) * 16 + p_ // 8
    b64 = np.zeros((128, B_TOTAL // 2), dtype=np.int32)
    b64[0:NSEL] = sel[:, b0.reshape(-1)]
    b64[64 : 64 + NSEL] = sel[:, b1.reshape(-1)]
    return [
        {
            "tslice": np.ascontiguousarray(tb16[:, c * JS : (c + 1) * JS, :]).view(
                ml_dtypes.bfloat16
            ),
            "bitsel": b64,
            "waddr": wa,
            "wpack": wp,
        }
        for c in range(N_CORES)
    ]


def _row_map():
    """Batch row for device out position (k, p, cc)."""
    k = np.arange(NCK)[:, None, None]
    p = np.arange(128)[None, :, None]
    cc = np.arange(CC)[None, None, :]
    j = cc * 128 + p
    tc = k * (CHUNK // 16) + j // 16
    r = j % 16
    half = tc // 1024
    u = tc % 1024
    g = u // 128
    m = half * 128 + u % 128
    return (g * 4096 + m * 16 + r).reshape(-1)


def _assemble(outs, inputs):
    dmap = _col_unperm()
    bmap = _row_map()
    res = []
    for o in outs:
        v = o["out"].reshape(B_TOTAL, JS)
        ro = np.empty_like(v)
        ro[bmap] = v
        res.append(ro[:, dmap])
    return np.concatenate(res, axis=1)


def kernel(bits, tables, positions):
    nc = _get_nc()
    in_maps = _make_in_maps(
        {"bits": bits, "tables": tables, "positions": positions}
    )
    res = run_bass_kernel_spmd(nc, in_maps, core_ids=list(range(N_CORES)))
    return _assemble(res.results, None)


# revision 49
# speedup vs baseline: 1.0033x; 1.0033x over previous
"""nn_HashMapper Trainium2 kernel (8 NeuronCores, Bass/Tile) — v8.

Contract: kernel(**inputs) takes the FULL unsharded inputs
(bits [32768,1024] i32, tables [3,1024,16384] f32, positions [3,14] i32)
and returns the FULL output [32768,1024] u8.

Sharding (hardcoded): neurons j (1024) split across 8 cores (128 each) so
tables are read exactly once system-wide; every core computes the full
batch's hash addresses locally from a replicated 42-column slice of bits
(no cross-device communication at all).

Architecture — 2-bit packed tables, bf16 byte-view inputs:
  - tslice ships as the high 2 bytes of each f32 (exact for 0/1 values).
  - Each core packs its 128 neurons' table values into 2-bit fields of u16
    words via PE matmuls (8 neurons/word, 16 words = 32B per address);
    packed rows are padded to 256B in DRAM and gathered bitcast to int64
    (elem 32 x i64, the cheapest legal gather shape).
  - Addresses: bitsel packs the 42 selected bit-columns for each half of
    the batch into partitions [0:42]/[64:106]; PE matmuls produce all
    32768 addresses; a wrapped DRAM round-trip (split per batch half so
    the first half unlocks early) builds the replicated index tiles.
  - Votes: gathered words of the 3 hash tables are summed as u16 (2-bit
    fields hold 0..3, no carries); majority = bit1 of each field,
    extracted with 4 tensor_scalar (shift, and-0x0101) passes emitting two
    output bytes per u16; the host inverse-permutes output columns.
  - Engine layout: SP: consts | interleaved tslice tiles | wrapped addr
    writes | PT1 | half the out writes.  ACT: interleaved tslice tiles |
    PT0/PT2 | idx reads | other out writes.  Pool: bitsel cast-loads |
    gathers.  DVE: addr/psum copies | votes + unpack.  PE: all matmuls.
"""

from contextlib import ExitStack

import numpy as np

import concourse.bass as bass
import concourse.bacc as bacc
import concourse.tile as tile
import concourse.mybir as mybir
from concourse.bass_utils import run_bass_kernel_spmd

F32 = mybir.dt.float32
BF16 = mybir.dt.bfloat16
I32 = mybir.dt.int32
I16 = mybir.dt.int16
U16 = mybir.dt.uint16
I64 = mybir.dt.int64
U8 = mybir.dt.uint8
A = mybir.AluOpType

N_BITS = 1024
NE = 16384
H = 3
JS = 128
B_TOTAL = 32768
N_CORES = 8
NSEL = H * 14  # 42 selected bit columns

NW = 16         # u16 words per packed row (8 neurons each, 2-bit fields)
ROW_U16 = 128   # padded row size in u16 (256B, gather elem floor)
CHUNK = 4096    # gather chunk (batch rows per dma_gather)
CC = CHUNK // 128
NCK = B_TOTAL // CHUNK  # 8
NBT = B_TOTAL // 128    # 256 address blocks
GC = B_TOTAL // 16      # 2048 wrapped idx columns


def _build(n_cores=N_CORES, nq=4):
    nc = bacc.Bacc(
        "TRN2", target_bir_lowering=False, num_devices=n_cores, num_swdge_queues=nq
    )
    tslice = nc.dram_tensor("tslice", [H, JS, NE], BF16, kind="ExternalInput")
    bitsel = nc.dram_tensor("bitsel", [128, B_TOTAL // 2], I32, kind="ExternalInput")
    waddr = nc.dram_tensor("waddr", [128, H], BF16, kind="ExternalInput")
    wpack = nc.dram_tensor("wpack", [JS, NW], BF16, kind="ExternalInput")
    out = nc.dram_tensor("out", [B_TOTAL, JS], U8, kind="ExternalOutput")

    # wrapped addresses, split per bitsel half: [h, half, 16, 1024] i16
    addrw = nc.dram_tensor("addrw", [H, 2, 16, GC // 2], I16)
    # packed table, padded rows: row (h, a) = PT[h, a, 0:NW] words + pad
    PT = nc.dram_tensor("PT", [H, NE, ROW_U16], U16)

    with tile.TileContext(nc) as tc, ExitStack() as ctx:
        const = ctx.enter_context(tc.tile_pool(name="const", bufs=1))
        psT = ctx.enter_context(tc.tile_pool(name="psT", bufs=4, space="PSUM"))
        psA = ctx.enter_context(tc.tile_pool(name="psA", bufs=2, space="PSUM"))
        sbT = ctx.enter_context(tc.tile_pool(name="sbT", bufs=2))
        sbG = ctx.enter_context(tc.tile_pool(name="sbG", bufs=1))

        wpk = const.tile([JS, NW], BF16)
        nc.sync.dma_start(wpk[:, :], wpack[:, :])
        wad = const.tile([128, H], BF16)
        nc.sync.dma_start(wad[:, :], waddr[:, :])

        # ---- bitsel cast-load (Pool, i32 -> bf16) + address matmuls (PE),
        #      pipelined per column half (m = 2*blk + g, so column half ch
        #      yields addr16 columns [ch*128, (ch+1)*128)); copies on Pool ----
        bsl = const.tile([128, B_TOTAL // 2], BF16)
        addr16 = const.tile([128, NBT, H], I16)
        for ch in range(2):
            nc.gpsimd.dma_start(
                bsl[:, ch * (B_TOTAL // 4) : (ch + 1) * (B_TOTAL // 4)],
                bitsel[:, ch * (B_TOTAL // 4) : (ch + 1) * (B_TOTAL // 4)],
            )
            pa = psA.tile([128, NBT // 2, H], F32, tag="addr")
            for i in range(NBT // 2):
                blk, g = ch * 64 + i // 2, i % 2
                nc.tensor.matmul(
                    pa[:, i, :],
                    bsl[64 * g : 64 * g + NSEL, blk * 128 : (blk + 1) * 128],
                    wad[64 * g : 64 * g + NSEL, :],
                    start=True,
                    stop=True,
                )
            nc.gpsimd.tensor_copy(
                addr16[:, ch * (NBT // 2) : (ch + 1) * (NBT // 2), :],
                pa[:, :, :],
            )

        # wrapped-layout writeback per (h, bitsel-half).  The host orders
        # bitsel columns so matmul m's output partition p computes batch row
        # b = (p%8)*4096 + m*16 + p//8; region (h, half) holds slots
        # [r=p//8][(p%8)*128 + m%128] so each half lands as soon as its
        # bitsel half is processed.
        def wrap_write(eng, h, half):
            eng.dma_start(
                bass.AP(
                    addrw,
                    (h * 2 + half) * 16 * (GC // 2),
                    [[GC // 2, 16], [128, 8], [1, 128]],
                ),
                addr16[:, half * 128 : (half + 1) * 128, h],
            )

        idxs = [const.tile([128, GC], I16, tag=f"idx{h}", name=f"idx{h}")
                for h in range(H)]

        def idx_read(eng, h, half):
            eng.dma_start(
                idxs[h][:, half * (GC // 2) : (half + 1) * (GC // 2)],
                bass.AP(
                    addrw,
                    (h * 2 + half) * 16 * (GC // 2),
                    [[0, 8], [GC // 2, 16], [1, GC // 2]],
                ),
            )

        # ---- tslice load + pack (PE matmuls) + PT writes ----
        # Emission order is engine-queue order; interleave the addr-path
        # DMAs at queue positions where their dependencies are ready.
        TL = 2048  # tslice tile columns (addr)
        NT = NE // TL  # 8 tiles per h
        pts = [const.tile([128, NE // 128, NW], U16, name=f"pts{h}") for h in range(H)]

        def load_tile(h, t, eng):
            tsl = sbT.tile([128, TL], BF16, tag=f"tsl{t % 2}", bufs=3,
                           name=f"tsl_{h}_{t}")
            eng.dma_start(tsl[:, :], tslice[h, :, t * TL : (t + 1) * TL])
            ps = psT.tile([128, 16, NW], F32, tag="pack")
            for b16 in range(16):
                nc.tensor.matmul(
                    ps[:, b16, :],
                    tsl[:, b16 * 128 : (b16 + 1) * 128],
                    wpk[:, :],
                    start=True,
                    stop=True,
                )
            nc.vector.tensor_copy(pts[h][:, t * 16 : t * 16 + 16, :], ps[:, :, :])

        def pt_write(h, eng, q=None):
            # quarter-granular PT writes so each lands as soon as its pack
            # copies finish (rank range q*32:(q+1)*32)
            o, n = (0, NE // 128) if q is None else (q * 32, 32)
            eng.dma_start(
                bass.AP(
                    PT,
                    h * NE * ROW_U16 + o * 128 * ROW_U16,
                    [[ROW_U16, 128], [128 * ROW_U16, n], [1, NW]],
                ),
                pts[h][:, o : o + n, :],
            )

        # SP:  c | h0e×4 | h1e×2 | wrap-h0 | wrap-h1 | h1e×2 | h2e×4 |
        #      wrap-h2 | PT2
        # ACT: h0o×4 | h1o×2 | idx-h0 | h1o×2 | PT1 | idx-h1 | h2o×4 | idx-h2
        # Pool: bitsel | PT0 | gathers
        def load_pair(h, q):
            load_tile(h, 2 * q, nc.sync)
            load_tile(h, 2 * q + 1, nc.scalar)
            pt_write(h, nc.scalar if q % 2 == 0 else nc.sync, q)

        def load_h(h):
            for q in range(NT // 2):
                load_pair(h, q)

        load_h(0)
        wrap_write(nc.sync, 0, 0)
        idx_read(nc.sync, 0, 0)
        wrap_write(nc.scalar, 1, 0)
        idx_read(nc.scalar, 1, 0)
        load_pair(1, 0)
        load_pair(1, 1)
        wrap_write(nc.scalar, 2, 0)
        idx_read(nc.scalar, 2, 0)
        wrap_write(nc.sync, 0, 1)
        idx_read(nc.sync, 0, 1)
        load_pair(1, 2)
        load_pair(1, 3)
        wrap_write(nc.sync, 1, 1)
        idx_read(nc.sync, 1, 1)
        load_h(2)
        wrap_write(nc.scalar, 2, 1)
        idx_read(nc.scalar, 2, 1)

        # ---- gather + votes + out (hand-synced critical section) ----
        SH = [5, 4, 3]  # per-h gather slot counts
        gts = [
            [sbG.tile([128, CC, ROW_U16 // 4], I64, tag=f"g{h}s{s}", bufs=1,
                      name=f"g{h}s{s}") for s in range(SH[h])]
            for h in range(H)
        ]
        NVB = 3
        vts = [sbG.tile([128, CC, NW], U16, tag=f"vt{s}", bufs=1, name=f"vt{s}")
               for s in range(NVB)]
        ots = [sbG.tile([128, CC, 4, NW], U16, tag=f"os{s}", bufs=1, name=f"os{s}")
               for s in range(NVB)]
        # chunk table: 7 full chunks + 2 half chunks to shorten the final
        # vote+out tail (start batch-slot, rows)
        CHUNKS = [(i * CHUNK, CHUNK) for i in range(NCK - 1)]
        CHUNKS += [((NCK - 1) * CHUNK, CHUNK // 2),
                   ((NCK - 1) * CHUNK + CHUNK // 2, CHUNK // 2)]
        NCH = len(CHUNKS)

        def gather(h, k):
            start, n = CHUNKS[k]
            nc.gpsimd.dma_gather(
                gts[h][k % SH[h]][:, : n // 128, :],
                bass.AP(PT, h * NE * ROW_U16, [[ROW_U16, NE], [1, ROW_U16]]).bitcast(
                    I64
                ),
                idxs[h][:, start // 16 : (start + n) // 16],
                num_idxs=n,
                num_idxs_reg=n,
                elem_size=ROW_U16 // 4,
                single_packet=False,
                queue_num=0,
            )

        # staged issue order matched to feed readiness: h0 leads, then h1,
        # then h2 with the rest interleaved (tile auto-sync handles deps and
        # slot reuse; no barrier, so gathers overlap the prep phase)
        sched = [(0, k) for k in range(SH[0])]
        sched += [(1, k) for k in range(SH[1])]
        for k in range(NCH):
            sched.append((2, k))
            if k + SH[0] < NCH:
                sched.append((0, k + SH[0]))
            if k + SH[1] < NCH:
                sched.append((1, k + SH[1]))

        gath_emitted = [[False] * NCH for _ in range(H)]
        votes_done = [0]

        def emit_votes_up_to(kmax):
            # votes + unpack + out write for chunks [votes_done, kmax)
            for k in range(votes_done[0], kmax):
                start, n = CHUNKS[k]
                cc = n // 128
                g0 = gts[0][k % SH[0]][:, :cc, :].bitcast(U16)[:, :, 0:NW]
                g1 = gts[1][k % SH[1]][:, :cc, :].bitcast(U16)[:, :, 0:NW]
                g2 = gts[2][k % SH[2]][:, :cc, :].bitcast(U16)[:, :, 0:NW]
                vt = vts[k % NVB]
                nc.vector.tensor_tensor(vt[:, :cc, :], g0, g1, op=A.add)
                nc.vector.tensor_tensor(vt[:, :cc, :], vt[:, :cc, :], g2, op=A.add)
                ot = ots[k % NVB]
                for kk in range(4):
                    nc.vector.tensor_scalar(
                        ot[:, :cc, kk, :], vt[:, :cc, :], 2 * kk + 1, 0x0101,
                        op0=A.logical_shift_right, op1=A.bitwise_and,
                    )
                eng = nc.sync if k % 2 == 0 else nc.scalar
                eng.dma_start(
                    bass.AP(out, start * JS, [[cc * JS, 128], [1, cc * JS]]),
                    ots[k % NVB][:, :cc, :, :].bitcast(U8).rearrange(
                        "p cc x w -> p (cc x w)"
                    ),
                )
            votes_done[0] = max(votes_done[0], kmax)

        for h, k in sched:
            gather(h, k)
            gath_emitted[h][k] = True
            # emit vote chains as soon as all three gathers of a chunk exist
            kk = votes_done[0]
            while kk < NCH and all(gath_emitted[h2][kk] for h2 in range(H)):
                emit_votes_up_to(kk + 1)
                kk += 1
        emit_votes_up_to(NCH)

    nc.compile()
    return nc


def _make_weights(positions):
    """Host-side tiny weight tensors derived from positions."""
    import ml_dtypes

    waddr = np.zeros((128, H), dtype=np.float32)
    for h in range(H):
        for k in range(14):
            waddr[h * 14 + k, h] = float(1 << (13 - k))
            waddr[64 + h * 14 + k, h] = float(1 << (13 - k))
    wpack = np.zeros((JS, NW), dtype=np.float32)
    for jl in range(JS):
        wpack[jl, jl // 8] = float(4 ** (jl % 8))
    return (
        waddr.astype(ml_dtypes.bfloat16),
        wpack.astype(ml_dtypes.bfloat16),
    )


_NC_CACHE = {}


def _get_nc():
    if "nc" not in _NC_CACHE:
        _NC_CACHE["nc"] = _build()
    return _NC_CACHE["nc"]


OUT_NAMES = ["out"]


def _col_unperm():
    """Map output column j_local -> device column (k*32 + w*2 + byte)."""
    dmap = np.empty(JS, dtype=np.int64)
    for jl in range(JS):
        w, r = jl // 8, jl % 8
        k, b = r % 4, r // 4
        dmap[jl] = k * 32 + w * 2 + b
    return dmap


def _make_in_maps(inputs):
    import ml_dtypes

    bits = np.asarray(inputs["bits"], dtype=np.int32)
    tables = np.ascontiguousarray(np.asarray(inputs["tables"], dtype=np.float32))
    positions = np.asarray(inputs["positions"], dtype=np.int32)
    wa, wp = _make_weights(positions)
    cols = np.array(
        [N_BITS - 1 - positions[h, k] for h in range(H) for k in range(14)],
        dtype=np.int64,
    )
    # bf16 byte-view of the f32 tables (exact: values are 0.0/1.0)
    tb16 = tables.view(np.uint16)[:, :, 1::2]
    # bitsel: 42 selected columns; device column q = blk*128 + p carries
    # rows [0:42] for m = 2*blk (partition group 0) and rows [64:106] for
    # m = 2*blk + 1 (group 1); matmul m partition p computes batch row
    # b = (p%8)*4096 + m*16 + p//8 (see wrap_write)
    sel = bits[:, cols].T  # [42, B_TOTAL]
    blk_ = np.arange(128)[:, None]
    p_ = np.arange(128)[None, :]
    b0 = (p_ % 8) * 4096 + (2 * blk_) * 16 + p_ // 8  # [128, 128]
    b1 = (p_ % 8) * 4096 + (2 * blk_ + 1) * 16 + p_ // 8
    b64 = np.zeros((128, B_TOTAL // 2), dtype=np.int32)
    b64[0:NSEL] = sel[:, b0.reshape(-1)]
    b64[64 : 64 + NSEL] = sel[:, b1.reshape(-1)]
    return [
        {
            "tslice": np.ascontiguousarray(tb16[:, c * JS : (c + 1) * JS, :]).view(
                ml_dtypes.bfloat16
            ),
            "bitsel": b64,
            "waddr": wa,
            "wpack": wp,
        }
        for c in range(N_CORES)
    ]


def _chunk_table():
    chunks = [(i * CHUNK, CHUNK) for i in range(NCK - 1)]
    chunks += [((NCK - 1) * CHUNK, CHUNK // 2),
               ((NCK - 1) * CHUNK + CHUNK // 2, CHUNK // 2)]
    return chunks


def _row_map():
    """Batch row for each device-out row position."""
    bmap = np.empty(B_TOTAL, dtype=np.int64)
    for start, n in _chunk_table():
        cck = n // 128
        p = np.arange(128)[:, None]
        cc = np.arange(cck)[None, :]
        j = cc * 128 + p
        tc = start // 16 + j // 16
        r = j % 16
        half = tc // 1024
        u = tc % 1024
        g = u // 128
        m = half * 128 + u % 128
        b = g * 4096 + m * 16 + r
        bmap[start + (p * cck + cc).reshape(-1)] = b.reshape(-1)
    # bmap maps device row -> batch row
    return bmap


def _assemble(outs, inputs):
    dmap = _col_unperm()
    bmap = _row_map()
    res = []
    for o in outs:
        v = o["out"].reshape(B_TOTAL, JS)
        ro = np.empty_like(v)
        ro[bmap] = v
        res.append(ro[:, dmap])
    return np.concatenate(res, axis=1)


def kernel(bits, tables, positions):
    nc = _get_nc()
    in_maps = _make_in_maps(
        {"bits": bits, "tables": tables, "positions": positions}
    )
    res = run_bass_kernel_spmd(nc, in_maps, core_ids=list(range(N_CORES)))
    return _assemble(res.results, None)


# revision 51
# speedup vs baseline: 1.0318x; 1.0284x over previous
"""nn_HashMapper Trainium2 kernel (8 NeuronCores, Bass/Tile) — v8.

Contract: kernel(**inputs) takes the FULL unsharded inputs
(bits [32768,1024] i32, tables [3,1024,16384] f32, positions [3,14] i32)
and returns the FULL output [32768,1024] u8.

Sharding (hardcoded): neurons j (1024) split across 8 cores (128 each) so
tables are read exactly once system-wide; every core computes the full
batch's hash addresses locally from a replicated 42-column slice of bits
(no cross-device communication at all).

Architecture — 2-bit packed tables, bf16 byte-view inputs:
  - tslice ships as the high 2 bytes of each f32 (exact for 0/1 values).
  - Each core packs its 128 neurons' table values into 2-bit fields of u16
    words via PE matmuls (8 neurons/word, 16 words = 32B per address);
    packed rows are padded to 256B in DRAM and gathered bitcast to int64
    (elem 32 x i64, the cheapest legal gather shape).
  - Addresses: bitsel packs the 42 selected bit-columns for each half of
    the batch into partitions [0:42]/[64:106]; PE matmuls produce all
    32768 addresses; a wrapped DRAM round-trip (split per batch half so
    the first half unlocks early) builds the replicated index tiles.
  - Votes: gathered words of the 3 hash tables are summed as u16 (2-bit
    fields hold 0..3, no carries); majority = bit1 of each field,
    extracted with 4 tensor_scalar (shift, and-0x0101) passes emitting two
    output bytes per u16; the host inverse-permutes output columns.
  - Engine layout: SP: consts | interleaved tslice tiles | wrapped addr
    writes | PT1 | half the out writes.  ACT: interleaved tslice tiles |
    PT0/PT2 | idx reads | other out writes.  Pool: bitsel cast-loads |
    gathers.  DVE: addr/psum copies | votes + unpack.  PE: all matmuls.
"""

from contextlib import ExitStack

import numpy as np

import concourse.bass as bass
import concourse.bacc as bacc
import concourse.tile as tile
import concourse.mybir as mybir
from concourse.bass_utils import run_bass_kernel_spmd

F32 = mybir.dt.float32
BF16 = mybir.dt.bfloat16
I32 = mybir.dt.int32
I16 = mybir.dt.int16
U16 = mybir.dt.uint16
I64 = mybir.dt.int64
U8 = mybir.dt.uint8
A = mybir.AluOpType

N_BITS = 1024
NE = 16384
H = 3
JS = 128
B_TOTAL = 32768
N_CORES = 8
NSEL = H * 14  # 42 selected bit columns

NW = 16         # u16 words per packed row (8 neurons each, 2-bit fields)
ROW_U16 = 128   # padded row size in u16 (256B, gather elem floor)
CHUNK = 4096    # gather chunk (batch rows per dma_gather)
CC = CHUNK // 128
NCK = B_TOTAL // CHUNK  # 8
NBT = B_TOTAL // 128    # 256 address blocks
GC = B_TOTAL // 16      # 2048 wrapped idx columns


def _build(n_cores=N_CORES, nq=4):
    nc = bacc.Bacc(
        "TRN2", target_bir_lowering=False, num_devices=n_cores, num_swdge_queues=nq
    )
    tslice = nc.dram_tensor("tslice", [H, JS, NE], BF16, kind="ExternalInput")
    bitsel = nc.dram_tensor("bitsel", [128, B_TOTAL // 2], I32, kind="ExternalInput")
    waddr = nc.dram_tensor("waddr", [128, H], BF16, kind="ExternalInput")
    wpack = nc.dram_tensor("wpack", [JS, NW], BF16, kind="ExternalInput")
    out = nc.dram_tensor("out", [B_TOTAL, JS], U8, kind="ExternalOutput")

    # wrapped addresses, split per bitsel half: [h, half, 16, 1024] i16
    addrw = nc.dram_tensor("addrw", [H, 2, 16, GC // 2], I16)
    # packed table, padded rows: row (h, a) = PT[h, a, 0:NW] words + pad
    PT = nc.dram_tensor("PT", [H, NE, ROW_U16], U16)

    with tile.TileContext(nc) as tc, ExitStack() as ctx:
        const = ctx.enter_context(tc.tile_pool(name="const", bufs=1))
        psT = ctx.enter_context(tc.tile_pool(name="psT", bufs=4, space="PSUM"))
        psA = ctx.enter_context(tc.tile_pool(name="psA", bufs=2, space="PSUM"))
        sbT = ctx.enter_context(tc.tile_pool(name="sbT", bufs=2))
        sbG = ctx.enter_context(tc.tile_pool(name="sbG", bufs=1))

        wpk = const.tile([JS, NW], BF16)
        nc.sync.dma_start(wpk[:, :], wpack[:, :])
        wad = const.tile([128, H], BF16)
        nc.sync.dma_start(wad[:, :], waddr[:, :])

        # ---- bitsel cast-load (Pool, i32 -> bf16) + address matmuls (PE),
        #      pipelined per column half (m = 2*blk + g, so column half ch
        #      yields addr16 columns [ch*128, (ch+1)*128)); copies on Pool ----
        bsl = const.tile([128, B_TOTAL // 2], BF16)
        addr16 = const.tile([128, NBT, H], I16)
        for ch in range(2):
            nc.gpsimd.dma_start(
                bsl[:, ch * (B_TOTAL // 4) : (ch + 1) * (B_TOTAL // 4)],
                bitsel[:, ch * (B_TOTAL // 4) : (ch + 1) * (B_TOTAL // 4)],
            )
            pa = psA.tile([128, NBT // 2, H], F32, tag="addr")
            for i in range(NBT // 2):
                blk, g = ch * 64 + i // 2, i % 2
                nc.tensor.matmul(
                    pa[:, i, :],
                    bsl[64 * g : 64 * g + NSEL, blk * 128 : (blk + 1) * 128],
                    wad[64 * g : 64 * g + NSEL, :],
                    start=True,
                    stop=True,
                )
            nc.gpsimd.tensor_copy(
                addr16[:, ch * (NBT // 2) : (ch + 1) * (NBT // 2), :],
                pa[:, :, :],
            )

        # wrapped-layout writeback per (h, bitsel-half).  The host orders
        # bitsel columns so matmul m's output partition p computes batch row
        # b = (p%8)*4096 + m*16 + p//8; region (h, half) holds slots
        # [r=p//8][(p%8)*128 + m%128] so each half lands as soon as its
        # bitsel half is processed.
        def wrap_write(eng, h, half):
            eng.dma_start(
                bass.AP(
                    addrw,
                    (h * 2 + half) * 16 * (GC // 2),
                    [[GC // 2, 16], [128, 8], [1, 128]],
                ),
                addr16[:, half * 128 : (half + 1) * 128, h],
            )

        idxs = [const.tile([128, GC], I16, tag=f"idx{h}", name=f"idx{h}")
                for h in range(H)]

        def idx_read(eng, h, half):
            eng.dma_start(
                idxs[h][:, half * (GC // 2) : (half + 1) * (GC // 2)],
                bass.AP(
                    addrw,
                    (h * 2 + half) * 16 * (GC // 2),
                    [[0, 8], [GC // 2, 16], [1, GC // 2]],
                ),
            )

        # ---- tslice load + pack (PE matmuls) + PT writes ----
        # Emission order is engine-queue order; interleave the addr-path
        # DMAs at queue positions where their dependencies are ready.
        TL = 2048  # tslice tile columns (addr)
        NT = NE // TL  # 8 tiles per h
        pts = [const.tile([128, NE // 128, NW], U16, name=f"pts{h}") for h in range(H)]

        def load_tile(h, t, eng):
            tsl = sbT.tile([128, TL], BF16, tag=f"tsl{t % 2}", bufs=3,
                           name=f"tsl_{h}_{t}")
            eng.dma_start(tsl[:, :], tslice[h, :, t * TL : (t + 1) * TL])
            ps = psT.tile([128, 16, NW], F32, tag="pack")
            for b16 in range(16):
                nc.tensor.matmul(
                    ps[:, b16, :],
                    tsl[:, b16 * 128 : (b16 + 1) * 128],
                    wpk[:, :],
                    start=True,
                    stop=True,
                )
            nc.vector.tensor_copy(pts[h][:, t * 16 : t * 16 + 16, :], ps[:, :, :])

        def pt_write(h, eng, q=None):
            # quarter-granular PT writes so each lands as soon as its pack
            # copies finish (rank range q*32:(q+1)*32)
            o, n = (0, NE // 128) if q is None else (q * 32, 32)
            eng.dma_start(
                bass.AP(
                    PT,
                    h * NE * ROW_U16 + o * 128 * ROW_U16,
                    [[ROW_U16, 128], [128 * ROW_U16, n], [1, NW]],
                ),
                pts[h][:, o : o + n, :],
            )

        # SP:  c | h0e×4 | h1e×2 | wrap-h0 | wrap-h1 | h1e×2 | h2e×4 |
        #      wrap-h2 | PT2
        # ACT: h0o×4 | h1o×2 | idx-h0 | h1o×2 | PT1 | idx-h1 | h2o×4 | idx-h2
        # Pool: bitsel | PT0 | gathers
        def load_pair(h, q):
            load_tile(h, 2 * q, nc.sync)
            load_tile(h, 2 * q + 1, nc.scalar)
            pt_write(h, nc.scalar if q % 2 == 0 else nc.sync, q)

        def load_h(h):
            for q in range(NT // 2):
                load_pair(h, q)

        load_h(0)
        wrap_write(nc.sync, 0, 0)
        idx_read(nc.sync, 0, 0)
        wrap_write(nc.scalar, 1, 0)
        idx_read(nc.scalar, 1, 0)
        load_pair(1, 0)
        load_pair(1, 1)
        wrap_write(nc.scalar, 2, 0)
        idx_read(nc.scalar, 2, 0)
        wrap_write(nc.sync, 0, 1)
        idx_read(nc.sync, 0, 1)
        load_pair(1, 2)
        load_pair(1, 3)
        wrap_write(nc.sync, 1, 1)
        idx_read(nc.sync, 1, 1)
        load_h(2)
        wrap_write(nc.scalar, 2, 1)
        idx_read(nc.scalar, 2, 1)

        # ---- gather + votes + out (hand-synced critical section) ----
        SH = [5, 4, 3]  # per-h gather slot counts
        gts = [
            [sbG.tile([128, CC, ROW_U16 // 4], I64, tag=f"g{h}s{s}", bufs=1,
                      name=f"g{h}s{s}") for s in range(SH[h])]
            for h in range(H)
        ]
        NVB = 3
        vts = [sbG.tile([128, CC, NW], U16, tag=f"vt{s}", bufs=1, name=f"vt{s}")
               for s in range(NVB)]
        ots = [sbG.tile([128, CC, 4, NW], U16, tag=f"os{s}", bufs=1, name=f"os{s}")
               for s in range(NVB)]
        # chunk table (start batch-slot, rows)
        CHUNKS = [(i * CHUNK, CHUNK) for i in range(NCK)]
        NCH = len(CHUNKS)

        def gather(h, k):
            start, n = CHUNKS[k]
            nc.gpsimd.dma_gather(
                gts[h][k % SH[h]][:, : n // 128, :],
                bass.AP(PT, h * NE * ROW_U16, [[ROW_U16, NE], [1, ROW_U16]]).bitcast(
                    I64
                ),
                idxs[h][:, start // 16 : (start + n) // 16],
                num_idxs=n,
                num_idxs_reg=n,
                elem_size=ROW_U16 // 4,
                single_packet=False,
                queue_num=0,
            )

        # staged issue order matched to feed readiness: h0 leads, then h1,
        # then h2 with the rest interleaved (tile auto-sync handles deps and
        # slot reuse; no barrier, so gathers overlap the prep phase)
        sched = [(0, k) for k in range(SH[0])]
        sched += [(1, k) for k in range(SH[1])]
        for k in range(NCH):
            sched.append((2, k))
            if k + SH[0] < NCH:
                sched.append((0, k + SH[0]))
            if k + SH[1] < NCH:
                sched.append((1, k + SH[1]))

        gath_emitted = [[False] * NCH for _ in range(H)]
        votes_done = [0]

        def emit_votes_up_to(kmax):
            # votes + unpack + out write for chunks [votes_done, kmax)
            for k in range(votes_done[0], kmax):
                start, n = CHUNKS[k]
                cc = n // 128
                g0 = gts[0][k % SH[0]][:, :cc, :].bitcast(U16)[:, :, 0:NW]
                g1 = gts[1][k % SH[1]][:, :cc, :].bitcast(U16)[:, :, 0:NW]
                g2 = gts[2][k % SH[2]][:, :cc, :].bitcast(U16)[:, :, 0:NW]
                vt = vts[k % NVB]
                nc.vector.tensor_tensor(vt[:, :cc, :], g0, g1, op=A.add)
                nc.vector.tensor_tensor(vt[:, :cc, :], vt[:, :cc, :], g2, op=A.add)
                ot = ots[k % NVB]
                for kk in range(4):
                    nc.vector.tensor_scalar(
                        ot[:, :cc, kk, :], vt[:, :cc, :], 2 * kk + 1, 0x0101,
                        op0=A.logical_shift_right, op1=A.bitwise_and,
                    )
                eng = nc.sync if k % 2 == 0 else nc.scalar
                eng.dma_start(
                    bass.AP(out, start * JS, [[cc * JS, 128], [1, cc * JS]]),
                    ots[k % NVB][:, :cc, :, :].bitcast(U8).rearrange(
                        "p cc x w -> p (cc x w)"
                    ),
                )
            votes_done[0] = max(votes_done[0], kmax)

        for h, k in sched:
            gather(h, k)
            gath_emitted[h][k] = True
            # emit vote chains as soon as all three gathers of a chunk exist
            kk = votes_done[0]
            while kk < NCH and all(gath_emitted[h2][kk] for h2 in range(H)):
                emit_votes_up_to(kk + 1)
                kk += 1
        emit_votes_up_to(NCH)

    nc.compile()
    return nc


def _make_weights(positions):
    """Host-side tiny weight tensors derived from positions."""
    import ml_dtypes

    waddr = np.zeros((128, H), dtype=np.float32)
    for h in range(H):
        for k in range(14):
            waddr[h * 14 + k, h] = float(1 << (13 - k))
            waddr[64 + h * 14 + k, h] = float(1 << (13 - k))
    wpack = np.zeros((JS, NW), dtype=np.float32)
    for jl in range(JS):
        wpack[jl, jl // 8] = float(4 ** (jl % 8))
    return (
        waddr.astype(ml_dtypes.bfloat16),
        wpack.astype(ml_dtypes.bfloat16),
    )


_NC_CACHE = {}


def _get_nc():
    if "nc" not in _NC_CACHE:
        _NC_CACHE["nc"] = _build()
    return _NC_CACHE["nc"]


OUT_NAMES = ["out"]


def _col_unperm():
    """Map output column j_local -> device column (k*32 + w*2 + byte)."""
    dmap = np.empty(JS, dtype=np.int64)
    for jl in range(JS):
        w, r = jl // 8, jl % 8
        k, b = r % 4, r // 4
        dmap[jl] = k * 32 + w * 2 + b
    return dmap


def _make_in_maps(inputs):
    import ml_dtypes

    bits = np.asarray(inputs["bits"], dtype=np.int32)
    tables = np.ascontiguousarray(np.asarray(inputs["tables"], dtype=np.float32))
    positions = np.asarray(inputs["positions"], dtype=np.int32)
    wa, wp = _make_weights(positions)
    cols = np.array(
        [N_BITS - 1 - positions[h, k] for h in range(H) for k in range(14)],
        dtype=np.int64,
    )
    # bf16 byte-view of the f32 tables (exact: values are 0.0/1.0)
    tb16 = tables.view(np.uint16)[:, :, 1::2]
    # bitsel: 42 selected columns; device column q = blk*128 + p carries
    # rows [0:42] for m = 2*blk (partition group 0) and rows [64:106] for
    # m = 2*blk + 1 (group 1); matmul m partition p computes batch row
    # b = (p%8)*4096 + m*16 + p//8 (see wrap_write)
    sel = bits[:, cols].T  # [42, B_TOTAL]
    blk_ = np.arange(128)[:, None]
    p_ = np.arange(128)[None, :]
    b0 = (p_ % 8) * 4096 + (2 * blk_) * 16 + p_ // 8  # [128, 128]
    b1 = (p_ % 8) * 4096 + (2 * blk_ + 1) * 16 + p_ // 8
    b64 = np.zeros((128, B_TOTAL // 2), dtype=np.int32)
    b64[0:NSEL] = sel[:, b0.reshape(-1)]
    b64[64 : 64 + NSEL] = sel[:, b1.reshape(-1)]
    return [
        {
            "tslice": np.ascontiguousarray(tb16[:, c * JS : (c + 1) * JS, :]).view(
                ml_dtypes.bfloat16
            ),
            "bitsel": b64,
            "waddr": wa,
            "wpack": wp,
        }
        for c in range(N_CORES)
    ]


def _chunk_table():
    return [(i * CHUNK, CHUNK) for i in range(NCK)]


def _row_map():
    """Batch row for each device-out row position."""
    bmap = np.empty(B_TOTAL, dtype=np.int64)
    for start, n in _chunk_table():
        cck = n // 128
        p = np.arange(128)[:, None]
        cc = np.arange(cck)[None, :]
        j = cc * 128 + p
        tc = start // 16 + j // 16
        r = j % 16
        half = tc // 1024
        u = tc % 1024
        g = u // 128
        m = half * 128 + u % 128
        b = g * 4096 + m * 16 + r
        bmap[start + (p * cck + cc).reshape(-1)] = b.reshape(-1)
    # bmap maps device row -> batch row
    return bmap


def _assemble(outs, inputs):
    dmap = _col_unperm()
    bmap = _row_map()
    res = []
    for o in outs:
        v = o["out"].reshape(B_TOTAL, JS)
        ro = np.empty_like(v)
        ro[bmap] = v
        res.append(ro[:, dmap])
    return np.concatenate(res, axis=1)


def kernel(bits, tables, positions):
    nc = _get_nc()
    in_maps = _make_in_maps(
        {"bits": bits, "tables": tables, "positions": positions}
    )
    res = run_bass_kernel_spmd(nc, in_maps, core_ids=list(range(N_CORES)))
    return _assemble(res.results, None)
